# revision 7
# baseline (speedup 1.0000x reference)
"""GQA attention block (B=1, T=2048, HID=2048, NQ=16, NKV=8, D=128) on 8 TRN2
NeuronCores.

Sharding: tensor-parallel over heads. Core c owns q-heads {2c, 2c+1} and
kv-head c. Each core computes, from the full x:
  Q^T/K^T/V^T shards (transposed layouts, d on partitions)  ->  per-head
  RMSNorm + RoPE  ->  causal softmax attention (no max-subtraction; scores
  are O(5) for RMS-normed q/k)  ->  partial o_proj with Wo row-shard.
The 8 partial [T, HID] outputs are summed on the host (the row-parallel
"unshard" step).

All matmuls run as float32r (full PE rate at free-dim 512, ~1e-4 rel err).
"""

import os
import sys

sys.path.insert(0, "/opt/trn_rl_repo")

import numpy as np

import concourse.bass as bass  # noqa: F401  (bass must import before tile)
import concourse.mybir as mybir
import concourse.tile as tile
from concourse import bacc
from concourse.bass_utils import run_bass_kernel_spmd
from concourse.masks import make_identity

N_CORES = 8
T = 2048
HID = 2048
NQ, NKV, D = 16, 8, 128
HQ = NQ // N_CORES  # q heads per core = 2
EPS = 1e-6
SCALE = D**-0.5

P = 128
NK = HID // P       # 16 k-chunks for projections
NTR = T // 512      # 4 T-ranges of 512
NTT = T // P        # 16 T-tiles of 128

F32 = mybir.dt.float32
F32R = mybir.dt.float32r
ACT_EXP = mybir.ActivationFunctionType.Exp
ACT_SQUARE = mybir.ActivationFunctionType.Square
ACT_SQRT = mybir.ActivationFunctionType.Sqrt


def build_nc():
    nc = bacc.Bacc("TRN2", target_bir_lowering=False, debug=False,
                   num_devices=N_CORES)

    # ---- DRAM tensors (names = in_map keys) ----
    xt = nc.dram_tensor("xt", [HID, T], F32R, kind="ExternalInput")
    wq = nc.dram_tensor("wq", [HID, HQ * D], F32R, kind="ExternalInput")
    wk = nc.dram_tensor("wk", [HID, D], F32R, kind="ExternalInput")
    wv = nc.dram_tensor("wv", [HID, D], F32R, kind="ExternalInput")
    wo = nc.dram_tensor("wo", [HQ * D, HID], F32R, kind="ExternalInput")
    cosT = nc.dram_tensor("cosT", [D, T], F32, kind="ExternalInput")
    sinT = nc.dram_tensor("sinT", [D, T], F32, kind="ExternalInput")
    qw = nc.dram_tensor("qw", [D, 1], F32, kind="ExternalInput")
    kw = nc.dram_tensor("kw", [D, 1], F32, kind="ExternalInput")
    masks = nc.dram_tensor("masks", [P, 4, 512], F32, kind="ExternalInput")
    out = nc.dram_tensor("out", [T, HID], F32, kind="ExternalOutput")

    with tile.TileContext(nc) as tc:
        with (
            tc.tile_pool(name="cst", bufs=1) as cst,
            tc.tile_pool(name="fin", bufs=1) as fin,
        ):
            # ---------- constants / weights resident in SBUF ----------
            wq_sb = cst.tile([P, NK, HQ * D], F32R)
            wk_sb = cst.tile([P, NK, D], F32R)
            wv_sb = cst.tile([P, NK, D], F32R)
            nc.sync.dma_start(wq_sb[:], wq[:].rearrange("(k p) c -> p k c", p=P))
            nc.sync.dma_start(wk_sb[:], wk[:].rearrange("(k p) c -> p k c", p=P))
            nc.sync.dma_start(wv_sb[:], wv[:].rearrange("(k p) c -> p k c", p=P))
            masks_sb = cst.tile([P, 4, 512], F32)
            nc.sync.dma_start(masks_sb[:], masks[:])
            qw_sb = cst.tile([P, 1], F32)
            kw_sb = cst.tile([P, 1], F32)
            nc.sync.dma_start(qw_sb[:], qw[:])
            nc.sync.dma_start(kw_sb[:], kw[:])
            ones_f = cst.tile([P, 1], F32)
            nc.vector.memset(ones_f[:], 1.0)
            eps_sb = cst.tile([1, 1], F32)
            nc.vector.memset(eps_sb[:], EPS)
            ones_r = cst.tile([P, 1], F32R)
            nc.scalar.copy(ones_r[:], ones_f[:])
            ident = cst.tile([P, P], F32)
            make_identity(nc, ident[:])

            # final (post RMS+RoPE) transposed activations, f32r
            qT = [fin.tile([P, T], F32R, name=f"qT{h}") for h in range(HQ)]
            kT = fin.tile([P, T], F32R)
            vnat = fin.tile([P, NTT, D], F32R)  # [kv-tile part, tile idx, d]

            with tc.tile_pool(name="rawp", bufs=1) as rawp:
                raw = {
                    "q0": rawp.tile([P, T], F32, name="raw_q0"),
                    "q1": rawp.tile([P, T], F32, name="raw_q1"),
                    "k": rawp.tile([P, T], F32, name="raw_k"),
                    "v": rawp.tile([P, T], F32, name="raw_v"),
                }
                # ============= Phase A: QKV projection (transposed) ======
                CB = [("q0", 0), ("q1", 1), ("k", 2), ("v", 3)]
                with (
                    tc.tile_pool(name="xtp", bufs=2) as xtp,
                    tc.tile_pool(name="psA", bufs=5, space="PSUM") as psA,
                ):
                    for tr in range(NTR):
                        ts = slice(tr * 512, (tr + 1) * 512)
                        xch = xtp.tile([P, NK, 512], F32R, name="xch")
                        nc.sync.dma_start(
                            xch[:],
                            xt[:].rearrange("(k p) t -> p k t", p=P)[:, :, ts],
                        )
                        for name, cb in CB:
                            ps = psA.tile([P, 512], F32, name="psA_t")
                            for k in range(NK):
                                if cb < 2:
                                    lhsT = wq_sb[:, k, cb * D:(cb + 1) * D]
                                elif cb == 2:
                                    lhsT = wk_sb[:, k, :]
                                else:
                                    lhsT = wv_sb[:, k, :]
                                nc.tensor.matmul(
                                    ps[:], lhsT, xch[:, k, :],
                                    start=(k == 0), stop=(k == NK - 1),
                                )
                            nc.scalar.copy(raw[name][:, ts], ps[:])

                # ============= Phase B: RMSNorm + RoPE + V transpose =====
                with (
                    tc.tile_pool(name="rope", bufs=1) as rope,
                    tc.tile_pool(name="tmpp", bufs=1) as tmpp,
                    tc.tile_pool(name="psB", bufs=2, space="PSUM") as psB,
                ):
                    cos_sb = rope.tile([P, T], F32)
                    sin_sb = rope.tile([P, T], F32)
                    nc.sync.dma_start(cos_sb[:], cosT[:])
                    nc.sync.dma_start(sin_sb[:], sinT[:])
                    H = D // 2

                    for src, dst, w_sb in (
                        (raw["q0"], qT[0], qw_sb),
                        (raw["q1"], qT[1], qw_sb),
                        (raw["k"], kT, kw_sb),
                    ):
                        # rms over partition dim via ones-matmul of squares
                        sq = tmpp.tile([P, T], F32R, name="sq")
                        nc.scalar.activation(sq[:], src[:], ACT_SQUARE)
                        rstd = tmpp.tile([1, T], F32, name="rstd")
                        for r4 in range(NTR):
                            ts = slice(r4 * 512, (r4 + 1) * 512)
                            ssum = psB.tile([1, 512], F32, name="ssum")
                            nc.tensor.matmul(ssum[:], ones_r[:], sq[:, ts],
                                             start=True, stop=True)
                            # sqrt(mean + eps) as the psum eviction
                            nc.scalar.activation(rstd[:, ts], ssum[:],
                                                 ACT_SQRT, scale=1.0 / D,
                                                 bias=eps_sb[:])
                        nc.vector.reciprocal(rstd[:], rstd[:])
                        rinv_b = tmpp.tile([P, T], F32, name="rinv_b")
                        nc.gpsimd.partition_broadcast(rinv_b[:], rstd[:])
                        # normed = src * rinv * w   (w per-partition scale)
                        nq = tmpp.tile([P, T], F32, name="nq")
                        nc.vector.tensor_mul(nq[:], src[:], rinv_b[:])
                        nc.scalar.mul(nq[:], nq[:], w_sb[:])
                        # RoPE: dst = nq*cos -/+ shift(nq)*sin
                        # sin_sb holds sinT rolled by 64 partitions, so both
                        # DVE inputs share a base partition (HW constraint).
                        psn = tmpp.tile([P, T], F32, name="psn")
                        nc.vector.tensor_mul(psn[0:H, :], nq[H:D, :],
                                             sin_sb[H:D, :])
                        nc.vector.tensor_mul(psn[H:D, :], nq[0:H, :],
                                             sin_sb[0:H, :])
                        pc = tmpp.tile([P, T], F32, name="pc")
                        nc.vector.tensor_mul(pc[:], nq[:], cos_sb[:])
                        nc.vector.tensor_sub(dst[0:H, :], pc[0:H, :],
                                             psn[0:H, :])
                        nc.vector.tensor_add(dst[H:D, :], pc[H:D, :],
                                             psn[H:D, :])

                    # V: PE-transpose raw_v [128 d, T] -> vnat [kv, tile, d]
                    for st in range(NTT):
                        tp = psB.tile([P, P], F32, name="tp")
                        nc.tensor.transpose(
                            tp[:], raw["v"][:, st * P:(st + 1) * P], ident[:]
                        )
                        nc.scalar.copy(vnat[:, st, :], tp[:])

            with tc.tile_pool(name="ctxp", bufs=1) as ctxp:
                # per (head, q-range) normalized context, f32r
                ctxT = [
                    [ctxp.tile([P, 512], F32R, name=f"ctxT{h}_{qr}")
                     for qr in range(NTR)]
                    for h in range(HQ)
                ]
                # ================= Phase C: causal attention =============
                with (
                    tc.tile_pool(name="attp", bufs=3) as attp,
                    tc.tile_pool(name="psS", bufs=2, space="PSUM") as psS,
                    tc.tile_pool(name="psCX", bufs=2, space="PSUM") as psCX,
                    tc.tile_pool(name="psSM", bufs=2, space="PSUM") as psSM,
                ):
                    for h in range(HQ):
                        for qr in range(NTR):
                            qs = slice(qr * 512, (qr + 1) * 512)
                            n_st = 4 * (qr + 1)
                            ctx_ps = psCX.tile([P, 512], F32, name="ctx_ps")
                            sums_ps = psSM.tile([1, 512], F32, name="sums_ps")
                            for st in range(n_st):
                                s_ps = psS.tile([P, 512], F32, name="s_ps")
                                nc.tensor.matmul(
                                    s_ps[:], kT[:, st * P:(st + 1) * P],
                                    qT[h][:, qs], start=True, stop=True,
                                )
                                at = attp.tile([P, 512], F32R, name="at")
                                nc.scalar.activation(at[:], s_ps[:], ACT_EXP,
                                                     scale=SCALE)
                                if st >= 4 * qr:  # diagonal: causal mask
                                    j = st - 4 * qr
                                    nc.vector.tensor_mul(
                                        at[:], at[:].bitcast(F32),
                                        masks_sb[:, j, :],
                                    )
                                nc.tensor.matmul(
                                    ctx_ps[:], vnat[:, st, :], at[:],
                                    start=(st == 0), stop=(st == n_st - 1),
                                )
                                nc.tensor.matmul(
                                    sums_ps[:], ones_r[:], at[:],
                                    start=(st == 0), stop=(st == n_st - 1),
                                )
                            recip = attp.tile([1, 512], F32, name="recip")
                            nc.vector.reciprocal(recip[:], sums_ps[:])
                            rb = attp.tile([P, 512], F32, name="rb")
                            nc.gpsimd.partition_broadcast(rb[:], recip[:])
                            nc.vector.tensor_mul(ctxT[h][qr][:], ctx_ps[:],
                                                 rb[:])

                # ================= Phase D: partial o_proj ===============
                with (
                    tc.tile_pool(name="wop", bufs=1) as wop,
                    tc.tile_pool(name="outp", bufs=3) as outp,
                    tc.tile_pool(name="psD", bufs=3, space="PSUM") as psD,
                ):
                    wo_sb = wop.tile([P, HQ, HID], F32R)
                    nc.sync.dma_start(
                        wo_sb[:], wo[:].rearrange("(h p) n -> p h n", p=P)
                    )
                    for tt in range(NTT):
                        qr, off = tt // 4, (tt % 4) * P
                        for nr in range(NTR):
                            ns = slice(nr * 512, (nr + 1) * 512)
                            ps = psD.tile([P, 512], F32, name="psD_t")
                            for h in range(HQ):
                                nc.tensor.matmul(
                                    ps[:], ctxT[h][qr][:, off:off + P],
                                    wo_sb[:, h, ns],
                                    start=(h == 0), stop=(h == HQ - 1),
                                )
                            ot = outp.tile([P, 512], F32, name="ot")
                            nc.scalar.copy(ot[:], ps[:])
                            nc.sync.dma_start(
                                out[tt * P:(tt + 1) * P, ns], ot[:]
                            )

    nc.compile()
    return nc


_NC_CACHE = None


def get_nc():
    global _NC_CACHE
    if _NC_CACHE is None:
        _NC_CACHE = build_nc()
    return _NC_CACHE


def make_in_maps(x, cos, sin, Wq, Wk, Wv, Wo, q_norm_w, k_norm_w):
    x = np.asarray(x, dtype=np.float32).reshape(T, HID)
    xt = np.ascontiguousarray(x.T)
    cosT = np.ascontiguousarray(np.asarray(cos, np.float32).T)
    # rolled by 64: sinT_roll[d] = sin.T[(d - 64) % 128]
    sinT = np.ascontiguousarray(
        np.roll(np.asarray(sin, np.float32).T, 64, axis=0)
    )
    qw = np.ascontiguousarray(np.asarray(q_norm_w, np.float32).reshape(D, 1))
    kw = np.ascontiguousarray(np.asarray(k_norm_w, np.float32).reshape(D, 1))
    si = np.arange(P)[:, None, None]
    jj = np.arange(4)[None, :, None]
    qi = np.arange(512)[None, None, :]
    masks = (si + P * jj <= qi).astype(np.float32)
    Wq = np.asarray(Wq, np.float32)
    Wk = np.asarray(Wk, np.float32)
    Wv = np.asarray(Wv, np.float32)
    Wo = np.asarray(Wo, np.float32)
    in_maps = []
    for c in range(N_CORES):
        in_maps.append({
            "xt": xt,
            "wq": np.ascontiguousarray(Wq[:, c * HQ * D:(c + 1) * HQ * D]),
            "wk": np.ascontiguousarray(Wk[:, c * D:(c + 1) * D]),
            "wv": np.ascontiguousarray(Wv[:, c * D:(c + 1) * D]),
            "wo": np.ascontiguousarray(Wo[c * HQ * D:(c + 1) * HQ * D, :]),
            "cosT": cosT,
            "sinT": sinT,
            "qw": qw,
            "kw": kw,
            "masks": masks,
        })
    return in_maps


def kernel(x, cos, sin, Wq, Wk, Wv, Wo, q_norm_w, k_norm_w):
    nc = get_nc()
    in_maps = make_in_maps(x, cos, sin, Wq, Wk, Wv, Wo, q_norm_w, k_norm_w)
    res = run_bass_kernel_spmd(nc, in_maps, core_ids=list(range(N_CORES)))
    acc = np.zeros((T, HID), dtype=np.float32)
    for c in range(N_CORES):
        acc += res.results[c]["out"]
    return acc.reshape(1, T, HID)


# revision 35
# speedup vs baseline: 1.6064x; 1.6064x over previous
"""GQA attention block (B=1, T=2048, HID=2048, NQ=16, NKV=8, D=128) on 8 TRN2
NeuronCores.

Sharding: tensor-parallel over heads. Core c owns q-heads {2c, 2c+1} and
kv-head c. Each core computes, from the full x:
  Q^T/K^T/V^T shards (transposed layouts, d on partitions)  ->  per-head
  RMSNorm + RoPE  ->  causal softmax attention (no max-subtraction; scores
  are O(5) for RMS-normed q/k)  ->  partial o_proj with Wo row-shard.
The 8 partial [T, HID] outputs are summed on the host (the row-parallel
"unshard" step).

All matmuls run as float32r (full PE rate at free-dim 512, ~1e-4 rel err).
"""

import os
import sys

sys.path.insert(0, "/opt/trn_rl_repo")

import numpy as np

import concourse.bass as bass  # noqa: F401  (bass must import before tile)
import concourse.mybir as mybir
import concourse.tile as tile
from concourse import bacc
from concourse.bass_utils import run_bass_kernel_spmd
from concourse.masks import make_identity

N_CORES = 8
T = 2048
HID = 2048
NQ, NKV, D = 16, 8, 128
HQ = NQ // N_CORES  # q heads per core = 2
EPS = 1e-6
SCALE = D**-0.5

P = 128
NK = HID // P       # 16 k-chunks for projections
NTR = T // 512      # 4 T-ranges of 512
NTT = T // P        # 16 T-tiles of 128

F32 = mybir.dt.float32
F32R = mybir.dt.float32r
ACT_EXP = mybir.ActivationFunctionType.Exp
ACT_SQUARE = mybir.ActivationFunctionType.Square
ACT_SQRT = mybir.ActivationFunctionType.Sqrt


def build_nc():
    nc = bacc.Bacc("TRN2", target_bir_lowering=False, debug=False,
                   num_devices=N_CORES)

    # ---- DRAM tensors (names = in_map keys) ----
    xt = nc.dram_tensor("xt", [HID, T], F32R, kind="ExternalInput")
    wq = nc.dram_tensor("wq", [HID, HQ * D], F32R, kind="ExternalInput")
    wk = nc.dram_tensor("wk", [HID, D], F32R, kind="ExternalInput")
    wv = nc.dram_tensor("wv", [HID, D], F32R, kind="ExternalInput")
    wo = nc.dram_tensor("wo", [HQ * D, HID], F32R, kind="ExternalInput")
    cosT = nc.dram_tensor("cosT", [D, T], F32, kind="ExternalInput")
    sinT = nc.dram_tensor("sinT", [D, T], F32, kind="ExternalInput")
    qw = nc.dram_tensor("qw", [D, 1], F32, kind="ExternalInput")
    kw = nc.dram_tensor("kw", [D, 1], F32, kind="ExternalInput")
    masks = nc.dram_tensor("masks", [P, 4, 512], mybir.dt.bfloat16, kind="ExternalInput")
    out = nc.dram_tensor("out", [T, HID], F32, kind="ExternalOutput")

    with tile.TileContext(nc) as tc:
        with (
            tc.tile_pool(name="cst", bufs=1) as cst,
            tc.tile_pool(name="fin", bufs=1) as fin,
        ):
            # ---------- constants / weights resident in SBUF ----------
            wq_sb = cst.tile([P, NK, HQ * D], F32R)
            wk_sb = cst.tile([P, NK, D], F32R)
            wv_sb = cst.tile([P, NK, D], F32R)
            wq_r = wq[:].rearrange("(k p) c -> p k c", p=P)
            wk_r = wk[:].rearrange("(k p) c -> p k c", p=P)
            wv_r = wv[:].rearrange("(k p) c -> p k c", p=P)
            masks_sb = cst.tile([P, 4, 512], mybir.dt.bfloat16)
            qw_sb = cst.tile([P, 1], F32)
            kw_sb = cst.tile([P, 1], F32)
            nc.scalar.dma_start(qw_sb[:], qw[:])
            nc.scalar.dma_start(kw_sb[:], kw[:])
            ones_f = cst.tile([P, 1], F32)
            nc.vector.memset(ones_f[:], 1.0)
            eps_sb = cst.tile([1, 1], F32)
            nc.vector.memset(eps_sb[:], EPS)
            ones_r = cst.tile([P, 1], F32R)
            nc.scalar.copy(ones_r[:], ones_f[:])
            ident = cst.tile([P, P], F32)
            make_identity(nc, ident[:])

            # final (post RMS+RoPE) transposed activations, f32r
            qT = [fin.tile([P, T], F32R, name=f"qT{h}") for h in range(HQ)]
            kT = fin.tile([P, T], F32R)
            vnat = fin.tile([P, NTT, D], F32R)  # [kv-tile part, tile idx, d]

            with tc.tile_pool(name="rawp", bufs=1) as rawp:
                # ==== Phase A+B fused, per 512-wide T-slice ====
                # A: QKV projection matmuls (k-chunked DMAs so the first
                #    matmul starts after ~1/4 of the slice arrives).
                # B: RMSNorm + RoPE + V-transpose for the same slice, so
                #    attention inputs become ready slice-by-slice and
                #    phase C can start while later slices still project.
                CB = [("q0", 0), ("q1", 1), ("k", 2), ("v", 3)]
                KG = 4  # k-chunks per DMA group
                with (
                    tc.tile_pool(name="xtp", bufs=2) as xtp,
                    tc.tile_pool(name="rope", bufs=1) as rope,
                    tc.tile_pool(name="tmpp", bufs=1) as tmpp,
                    tc.tile_pool(name="psA", bufs=4, space="PSUM") as psA,
                    tc.tile_pool(name="psB", bufs=2, space="PSUM") as psB,
                ):
                    cos_sb = rope.tile([P, T], F32)
                    sin_sb = rope.tile([P, T], F32)
                    H = D // 2
                    xt_r = xt[:].rearrange("(k p) t -> p k t", p=P)

                    for tr in range(NTR):
                        ts = slice(tr * 512, (tr + 1) * 512)
                        raw = {
                            n: rawp.tile([P, 512], F32, name=f"raw_{n}")
                            for n, _ in CB
                        }
                        xch = xtp.tile([P, NK, 512], F32R, name="xch")
                        for kg in range(NK // KG):
                            ks = slice(kg * KG, (kg + 1) * KG)
                            if tr == 0:
                                nc.sync.dma_start(wq_sb[:, ks, :],
                                                  wq_r[:, ks, :])
                                nc.sync.dma_start(wk_sb[:, ks, :],
                                                  wk_r[:, ks, :])
                                nc.sync.dma_start(wv_sb[:, ks, :],
                                                  wv_r[:, ks, :])
                            nc.sync.dma_start(xch[:, ks, :], xt_r[:, ks, ts])
                        for name, cb in CB:
                            ps = psA.tile([P, 512], F32, name="psA_t")
                            for k in range(NK):
                                if cb < 2:
                                    lhsT = wq_sb[:, k, cb * D:(cb + 1) * D]
                                elif cb == 2:
                                    lhsT = wk_sb[:, k, :]
                                else:
                                    lhsT = wv_sb[:, k, :]
                                nc.tensor.matmul(
                                    ps[:], lhsT, xch[:, k, :],
                                    start=(k == 0), stop=(k == NK - 1),
                                )
                            nc.scalar.copy(raw[name][:], ps[:])

                        nc.scalar.dma_start(cos_sb[:, ts], cosT[:, ts])
                        nc.scalar.dma_start(sin_sb[:, ts], sinT[:, ts])
                        if tr == 0:
                            nc.scalar.dma_start(masks_sb[:], masks[:])
                        # ---- B for this slice ----
                        for src, dst, w_sb in (
                            (raw["q0"], qT[0], qw_sb),
                            (raw["q1"], qT[1], qw_sb),
                            (raw["k"], kT, kw_sb),
                        ):
                            sq = tmpp.tile([P, 512], F32R, name="sq")
                            nc.scalar.activation(sq[:], src[:], ACT_SQUARE)
                            ssum = psB.tile([1, 512], F32, name="ssum")
                            nc.tensor.matmul(ssum[:], ones_r[:], sq[:],
                                             start=True, stop=True)
                            rstd = tmpp.tile([1, 512], F32, name="rstd")
                            nc.scalar.activation(rstd[:], ssum[:], ACT_SQRT,
                                                 scale=1.0 / D, bias=eps_sb[:])
                            rinv = tmpp.tile([1, 512], F32, name="rinv")
                            nc.vector.reciprocal_approx_fast(rinv[:], rstd[:])
                            rinv_b = tmpp.tile([P, 512], F32, name="rinv_b")
                            nc.gpsimd.partition_broadcast(rinv_b[:], rinv[:])
                            nq = tmpp.tile([P, 512], F32, name="nq")
                            nc.vector.scalar_tensor_tensor(
                                nq[:], src[:], w_sb[:], rinv_b[:],
                                mybir.AluOpType.mult, mybir.AluOpType.mult,
                            )
                            # RoPE (sin_sb is pre-rolled by 64 partitions so
                            # both DVE inputs share a base partition)
                            psn = tmpp.tile([P, 512], F32, name="psn",
                                            tag="sq")
                            nc.vector.tensor_mul(psn[0:H, :], nq[H:D, :],
                                                 sin_sb[H:D, ts])
                            nc.vector.tensor_mul(psn[H:D, :], nq[0:H, :],
                                                 sin_sb[0:H, ts])
                            pc = tmpp.tile([P, 512], F32, name="pc", tag="rstd")
                            nc.vector.tensor_mul(pc[:], nq[:], cos_sb[:, ts])
                            nc.vector.tensor_sub(dst[0:H, ts], pc[0:H, :],
                                                 psn[0:H, :])
                            nc.vector.tensor_add(dst[H:D, ts], pc[H:D, :],
                                                 psn[H:D, :])

                        # V transposes for the 4 kv-tiles of this slice
                        for j in range(4):
                            st = 4 * tr + j
                            tp = psB.tile([P, P], F32, name="tp")
                            nc.tensor.transpose(
                                tp[:], raw["v"][:, j * P:(j + 1) * P],
                                ident[:],
                            )
                            nc.scalar.copy(vnat[:, st, :], tp[:])

            with (
                tc.tile_pool(name="ctxp", bufs=1) as ctxp,
                tc.tile_pool(name="wop", bufs=1) as wop,
            ):
                        nc.gpsimd.dma_start(
                    wo_sb[:], wo[:].rearrange("(h p) n -> p h n", p=P)
                )
                # per (head, q-range) normalized context, f32r
                ctxT = [
                    [ctxp.tile([P, 512], F32R, name=f"ctxT{h}_{qr}")
                     for qr in range(NTR)]
                    for h in range(HQ)
                ]
                # ================= Phase C: causal attention =============
                with (
                    tc.tile_pool(name="attp", bufs=3) as attp,
                    tc.tile_pool(name="atp", bufs=24) as atp,
                    tc.tile_pool(name="psS", bufs=4, space="PSUM") as psS,
                    tc.tile_pool(name="psCX", bufs=2, space="PSUM") as psCX,
                    tc.tile_pool(name="psSM", bufs=2, space="PSUM") as psSM,
                ):
                    for h in range(HQ):
                        for qr in range(NTR):
                            qs = slice(qr * 512, (qr + 1) * 512)
                            n_st = 4 * (qr + 1)
                            ats = []
                            for st in range(n_st):
                                s_ps = psS.tile([P, 512], F32, name="s_ps")
                                nc.tensor.matmul(
                                    s_ps[:], kT[:, st * P:(st + 1) * P],
                                    qT[h][:, qs], start=True, stop=True,
                                )
                                at = atp.tile([P, 512], F32R, name="at")
                                nc.scalar.activation(at[:], s_ps[:], ACT_EXP,
                                                     scale=SCALE)
                                if st >= 4 * qr:
                                    j = st - 4 * qr
                                    nc.vector.tensor_mul(
                                        at[:], at[:].bitcast(F32),
                                        masks_sb[:, j, :],
                                    )
                                ats.append(at)
                            ctx_ps = psCX.tile([P, 512], F32, name="ctx_ps")
                            sums_ps = psSM.tile([1, 512], F32,
                                                name="sums_ps")
                            for st in range(n_st):
                                nc.tensor.matmul(
                                    ctx_ps[:], vnat[:, st, :], ats[st][:],
                                    start=(st == 0), stop=(st == n_st - 1),
                                )
                                nc.tensor.matmul(
                                    sums_ps[:], ones_r[:], ats[st][:],
                                    start=(st == 0), stop=(st == n_st - 1),
                                )
                            recip = attp.tile([1, 512], F32, name="recip")
                            nc.vector.reciprocal_approx_fast(recip[:],
                                                             sums_ps[:])
                            rb = attp.tile([P, 512], F32, name="rb")
                            nc.gpsimd.partition_broadcast(rb[:], recip[:])
                            nc.vector.tensor_mul(ctxT[h][qr][:], ctx_ps[:],
                                                 rb[:])

                # ================= Phase D: partial o_proj ===============
                with (
                    tc.tile_pool(name="outp", bufs=8) as outp,
                    tc.tile_pool(name="psD", bufs=6, space="PSUM") as psD,
                ):
                    for tt in range(NTT):
                        qr, off = tt // 4, (tt % 4) * P
                        for nr in range(NTR):
                            ns = slice(nr * 512, (nr + 1) * 512)
                            ps = psD.tile([P, 512], F32, name="psD_t")
                            for h in range(HQ):
                                nc.tensor.matmul(
                                    ps[:], ctxT[h][qr][:, off:off + P],
                                    wo_sb[:, h, ns],
                                    start=(h == 0), stop=(h == HQ - 1),
                                )
                            ot = outp.tile([P, 512], F32, name="ot")
                            nc.vector.tensor_copy(ot[:], ps[:])
                            nc.sync.dma_start(
                                out[tt * P:(tt + 1) * P, ns], ot[:]
                            )

    nc.compile()
    return nc


_NC_CACHE = None


def get_nc():
    global _NC_CACHE
    if _NC_CACHE is None:
        _NC_CACHE = build_nc()
    return _NC_CACHE


def make_in_maps(x, cos, sin, Wq, Wk, Wv, Wo, q_norm_w, k_norm_w):
    x = np.asarray(x, dtype=np.float32).reshape(T, HID)
    xt = np.ascontiguousarray(x.T)
    cosT = np.ascontiguousarray(np.asarray(cos, np.float32).T)
    # rolled by 64: sinT_roll[d] = sin.T[(d - 64) % 128]
    sinT = np.ascontiguousarray(
        np.roll(np.asarray(sin, np.float32).T, 64, axis=0)
    )
    qw = np.ascontiguousarray(np.asarray(q_norm_w, np.float32).reshape(D, 1))
    kw = np.ascontiguousarray(np.asarray(k_norm_w, np.float32).reshape(D, 1))
    si = np.arange(P)[:, None, None]
    jj = np.arange(4)[None, :, None]
    qi = np.arange(512)[None, None, :]
    masks = (si + P * jj <= qi).astype(np.float32)
    Wq = np.asarray(Wq, np.float32)
    Wk = np.asarray(Wk, np.float32)
    Wv = np.asarray(Wv, np.float32)
    Wo = np.asarray(Wo, np.float32)
    in_maps = []
    for c in range(N_CORES):
        in_maps.append({
            "xt": xt,
            "wq": np.ascontiguousarray(Wq[:, c * HQ * D:(c + 1) * HQ * D]),
            "wk": np.ascontiguousarray(Wk[:, c * D:(c + 1) * D]),
            "wv": np.ascontiguousarray(Wv[:, c * D:(c + 1) * D]),
            "wo": np.ascontiguousarray(Wo[c * HQ * D:(c + 1) * HQ * D, :]),
            "cosT": cosT,
            "sinT": sinT,
            "qw": qw,
            "kw": kw,
            "masks": masks,
        })
    return in_maps


def kernel(x, cos, sin, Wq, Wk, Wv, Wo, q_norm_w, k_norm_w):
    nc = get_nc()
    in_maps = make_in_maps(x, cos, sin, Wq, Wk, Wv, Wo, q_norm_w, k_norm_w)
    res = run_bass_kernel_spmd(nc, in_maps, core_ids=list(range(N_CORES)))
    acc = np.zeros((T, HID), dtype=np.float32)
    for c in range(N_CORES):
        acc += res.results[c]["out"]
    return acc.reshape(1, T, HID)


# revision 36
# speedup vs baseline: 515.6453x; 320.9852x over previous
"""GQA attention block (B=1, T=2048, HID=2048, NQ=16, NKV=8, D=128) on 8 TRN2
NeuronCores.

Sharding: tensor-parallel over heads. Core c owns q-heads {2c, 2c+1} and
kv-head c. Each core computes, from the full x:
  Q^T/K^T/V^T shards (transposed layouts, d on partitions)  ->  per-head
  RMSNorm + RoPE  ->  causal softmax attention (no max-subtraction; scores
  are O(5) for RMS-normed q/k)  ->  partial o_proj with Wo row-shard.
The 8 partial [T, HID] outputs are summed on the host (the row-parallel
"unshard" step).

All matmuls run as float32r (full PE rate at free-dim 512, ~1e-4 rel err).
"""

import sys

sys.path.insert(0, "/opt/trn_rl_repo")

import numpy as np

import concourse.bass as bass  # noqa: F401  (bass must import before tile)
import concourse.mybir as mybir
import concourse.tile as tile
from concourse import bacc
from concourse.bass_utils import run_bass_kernel_spmd
from concourse.masks import make_identity

N_CORES = 8
T = 2048
HID = 2048
NQ, NKV, D = 16, 8, 128
HQ = NQ // N_CORES  # q heads per core = 2
EPS = 1e-6
SCALE = D**-0.5

P = 128
NK = HID // P       # 16 k-chunks for projections
NTR = T // 512      # 4 T-ranges of 512
NTT = T // P        # 16 T-tiles of 128

F32 = mybir.dt.float32
F32R = mybir.dt.float32r
ACT_EXP = mybir.ActivationFunctionType.Exp
ACT_SQUARE = mybir.ActivationFunctionType.Square
ACT_SQRT = mybir.ActivationFunctionType.Sqrt


def build_nc():
    nc = bacc.Bacc("TRN2", target_bir_lowering=False, debug=False,
                   num_devices=N_CORES)

    # ---- DRAM tensors (names = in_map keys) ----
    xt = nc.dram_tensor("xt", [HID, T], F32R, kind="ExternalInput")
    wq = nc.dram_tensor("wq", [HID, HQ * D], F32R, kind="ExternalInput")
    wk = nc.dram_tensor("wk", [HID, D], F32R, kind="ExternalInput")
    wv = nc.dram_tensor("wv", [HID, D], F32R, kind="ExternalInput")
    wo = nc.dram_tensor("wo", [HQ * D, HID], F32R, kind="ExternalInput")
    cosT = nc.dram_tensor("cosT", [D, T], F32, kind="ExternalInput")
    sinT = nc.dram_tensor("sinT", [D, T], F32, kind="ExternalInput")
    qw = nc.dram_tensor("qw", [D, 1], F32, kind="ExternalInput")
    kw = nc.dram_tensor("kw", [D, 1], F32, kind="ExternalInput")
    masks = nc.dram_tensor("masks", [P, 4, 512], mybir.dt.bfloat16, kind="ExternalInput")
    out = nc.dram_tensor("out", [T, HID], F32, kind="ExternalOutput")

    with tile.TileContext(nc) as tc:
        with (
            tc.tile_pool(name="cst", bufs=1) as cst,
            tc.tile_pool(name="fin", bufs=1) as fin,
        ):
            # ---------- constants / weights resident in SBUF ----------
            wq_sb = cst.tile([P, NK, HQ * D], F32R)
            wk_sb = cst.tile([P, NK, D], F32R)
            wv_sb = cst.tile([P, NK, D], F32R)
            wq_r = wq[:].rearrange("(k p) c -> p k c", p=P)
            wk_r = wk[:].rearrange("(k p) c -> p k c", p=P)
            wv_r = wv[:].rearrange("(k p) c -> p k c", p=P)
            masks_sb = cst.tile([P, 4, 512], mybir.dt.bfloat16)
            qw_sb = cst.tile([P, 1], F32)
            kw_sb = cst.tile([P, 1], F32)
            nc.scalar.dma_start(qw_sb[:], qw[:])
            nc.scalar.dma_start(kw_sb[:], kw[:])
            ones_f = cst.tile([P, 1], F32)
            nc.vector.memset(ones_f[:], 1.0)
            eps_sb = cst.tile([1, 1], F32)
            nc.vector.memset(eps_sb[:], EPS)
            ones_r = cst.tile([P, 1], F32R)
            nc.scalar.copy(ones_r[:], ones_f[:])
            ident = cst.tile([P, P], F32)
            make_identity(nc, ident[:])

            # final (post RMS+RoPE) transposed activations, f32r
            qT = [fin.tile([P, T], F32R, name=f"qT{h}") for h in range(HQ)]
            kT = fin.tile([P, T], F32R)
            vnat = fin.tile([P, NTT, D], F32R)  # [kv-tile part, tile idx, d]

            with tc.tile_pool(name="rawp", bufs=1) as rawp:
                # ==== Phase A+B fused, per 512-wide T-slice ====
                # A: QKV projection matmuls (k-chunked DMAs so the first
                #    matmul starts after ~1/4 of the slice arrives).
                # B: RMSNorm + RoPE + V-transpose for the same slice, so
                #    attention inputs become ready slice-by-slice and
                #    phase C can start while later slices still project.
                CB = [("q0", 0), ("q1", 1), ("k", 2), ("v", 3)]
                KG = 4  # k-chunks per DMA group
                with (
                    tc.tile_pool(name="xtp", bufs=2) as xtp,
                    tc.tile_pool(name="rope", bufs=1) as rope,
                    tc.tile_pool(name="tmpp", bufs=1) as tmpp,
                    tc.tile_pool(name="psA", bufs=4, space="PSUM") as psA,
                    tc.tile_pool(name="psB", bufs=2, space="PSUM") as psB,
                ):
                    cos_sb = rope.tile([P, T], F32)
                    sin_sb = rope.tile([P, T], F32)
                    H = D // 2
                    xt_r = xt[:].rearrange("(k p) t -> p k t", p=P)

                    for tr in range(NTR):
                        ts = slice(tr * 512, (tr + 1) * 512)
                        raw = {
                            n: rawp.tile([P, 512], F32, name=f"raw_{n}")
                            for n, _ in CB
                        }
                        xch = xtp.tile([P, NK, 512], F32R, name="xch")
                        for kg in range(NK // KG):
                            ks = slice(kg * KG, (kg + 1) * KG)
                            if tr == 0:
                                nc.sync.dma_start(wq_sb[:, ks, :],
                                                  wq_r[:, ks, :])
                                nc.sync.dma_start(wk_sb[:, ks, :],
                                                  wk_r[:, ks, :])
                                nc.sync.dma_start(wv_sb[:, ks, :],
                                                  wv_r[:, ks, :])
                            nc.sync.dma_start(xch[:, ks, :], xt_r[:, ks, ts])
                        for name, cb in CB:
                            ps = psA.tile([P, 512], F32, name="psA_t")
                            for k in range(NK):
                                if cb < 2:
                                    lhsT = wq_sb[:, k, cb * D:(cb + 1) * D]
                                elif cb == 2:
                                    lhsT = wk_sb[:, k, :]
                                else:
                                    lhsT = wv_sb[:, k, :]
                                nc.tensor.matmul(
                                    ps[:], lhsT, xch[:, k, :],
                                    start=(k == 0), stop=(k == NK - 1),
                                )
                            nc.scalar.copy(raw[name][:], ps[:])

                        nc.scalar.dma_start(cos_sb[:, ts], cosT[:, ts])
                        nc.scalar.dma_start(sin_sb[:, ts], sinT[:, ts])
                        if tr == 0:
                            nc.scalar.dma_start(masks_sb[:], masks[:])
                        # ---- B for this slice ----
                        for src, dst, w_sb in (
                            (raw["q0"], qT[0], qw_sb),
                            (raw["q1"], qT[1], qw_sb),
                            (raw["k"], kT, kw_sb),
                        ):
                            sq = tmpp.tile([P, 512], F32R, name="sq")
                            nc.scalar.activation(sq[:], src[:], ACT_SQUARE)
                            ssum = psB.tile([1, 512], F32, name="ssum")
                            nc.tensor.matmul(ssum[:], ones_r[:], sq[:],
                                             start=True, stop=True)
                            rstd = tmpp.tile([1, 512], F32, name="rstd")
                            nc.scalar.activation(rstd[:], ssum[:], ACT_SQRT,
                                                 scale=1.0 / D, bias=eps_sb[:])
                            rinv = tmpp.tile([1, 512], F32, name="rinv")
                            nc.vector.reciprocal_approx_fast(rinv[:], rstd[:])
                            rinv_b = tmpp.tile([P, 512], F32, name="rinv_b")
                            nc.gpsimd.partition_broadcast(rinv_b[:], rinv[:])
                            nq = tmpp.tile([P, 512], F32, name="nq")
                            nc.vector.scalar_tensor_tensor(
                                nq[:], src[:], w_sb[:], rinv_b[:],
                                mybir.AluOpType.mult, mybir.AluOpType.mult,
                            )
                            # RoPE (sin_sb is pre-rolled by 64 partitions so
                            # both DVE inputs share a base partition)
                            psn = tmpp.tile([P, 512], F32, name="psn",
                                            tag="sq")
                            nc.vector.tensor_mul(psn[0:H, :], nq[H:D, :],
                                                 sin_sb[H:D, ts])
                            nc.vector.tensor_mul(psn[H:D, :], nq[0:H, :],
                                                 sin_sb[0:H, ts])
                            pc = tmpp.tile([P, 512], F32, name="pc", tag="rstd")
                            nc.vector.tensor_mul(pc[:], nq[:], cos_sb[:, ts])
                            nc.vector.tensor_sub(dst[0:H, ts], pc[0:H, :],
                                                 psn[0:H, :])
                            nc.vector.tensor_add(dst[H:D, ts], pc[H:D, :],
                                                 psn[H:D, :])

                        # V transposes for the 4 kv-tiles of this slice
                        for j in range(4):
                            st = 4 * tr + j
                            tp = psB.tile([P, P], F32, name="tp")
                            nc.tensor.transpose(
                                tp[:], raw["v"][:, j * P:(j + 1) * P],
                                ident[:],
                            )
                            nc.scalar.copy(vnat[:, st, :], tp[:])

            with (
                tc.tile_pool(name="ctxp", bufs=1) as ctxp,
                tc.tile_pool(name="wop", bufs=1) as wop,
            ):
                        nc.gpsimd.dma_start(
                    wo_sb[:], wo[:].rearrange("(h p) n -> p h n", p=P)
                )
                # per (head, q-range) normalized context, f32r
                ctxT = [
                    [ctxp.tile([P, 512], F32R, name=f"ctxT{h}_{qr}")
                     for qr in range(NTR)]
                    for h in range(HQ)
                ]
                # ================= Phase C: causal attention =============
                with (
                    tc.tile_pool(name="attp", bufs=3) as attp,
                    tc.tile_pool(name="atp", bufs=24) as atp,
                    tc.tile_pool(name="psS", bufs=4, space="PSUM") as psS,
                    tc.tile_pool(name="psCX", bufs=2, space="PSUM") as psCX,
                    tc.tile_pool(name="psSM", bufs=2, space="PSUM") as psSM,
                ):
                    for h in range(HQ):
                        for qr in range(NTR):
                            qs = slice(qr * 512, (qr + 1) * 512)
                            n_st = 4 * (qr + 1)
                            ats = []
                            for st in range(n_st):
                                s_ps = psS.tile([P, 512], F32, name="s_ps")
                                nc.tensor.matmul(
                                    s_ps[:], kT[:, st * P:(st + 1) * P],
                                    qT[h][:, qs], start=True, stop=True,
                                )
                                at = atp.tile([P, 512], F32R, name="at")
                                nc.scalar.activation(at[:], s_ps[:], ACT_EXP,
                                                     scale=SCALE)
                                if st >= 4 * qr:
                                    j = st - 4 * qr
                                    nc.vector.tensor_mul(
                                        at[:], at[:].bitcast(F32),
                                        masks_sb[:, j, :],
                                    )
                                ats.append(at)
                            ctx_ps = psCX.tile([P, 512], F32, name="ctx_ps")
                            sums_ps = psSM.tile([1, 512], F32,
                                                name="sums_ps")
                            for st in range(n_st):
                                nc.tensor.matmul(
                                    ctx_ps[:], vnat[:, st, :], ats[st][:],
                                    start=(st == 0), stop=(st == n_st - 1),
                                )
                                nc.tensor.matmul(
                                    sums_ps[:], ones_r[:], ats[st][:],
                                    start=(st == 0), stop=(st == n_st - 1),
                                )
                            recip = attp.tile([1, 512], F32, name="recip")
                            nc.vector.reciprocal_approx_fast(recip[:],
                                                             sums_ps[:])
                            rb = attp.tile([P, 512], F32, name="rb")
                            nc.gpsimd.partition_broadcast(rb[:], recip[:])
                            nc.vector.tensor_mul(ctxT[h][qr][:], ctx_ps[:],
                                                 rb[:])

                # ================= Phase D: partial o_proj ===============
                with (
                    tc.tile_pool(name="outp", bufs=8) as outp,
                    tc.tile_pool(name="psD", bufs=6, space="PSUM") as psD,
                ):
                    for tt in range(NTT):
                        qr, off = tt // 4, (tt % 4) * P
                        for nr in range(NTR):
                            ns = slice(nr * 512, (nr + 1) * 512)
                            ps = psD.tile([P, 512], F32, name="psD_t")
                            for h in range(HQ):
                                nc.tensor.matmul(
                                    ps[:], ctxT[h][qr][:, off:off + P],
                                    wo_sb[:, h, ns],
                                    start=(h == 0), stop=(h == HQ - 1),
                                )
                            ot = outp.tile([P, 512], F32, name="ot")
                            nc.vector.tensor_copy(ot[:], ps[:])
                            nc.sync.dma_start(
                                out[tt * P:(tt + 1) * P, ns], ot[:]
                            )

    nc.compile()
    return nc


_NC_CACHE = None


def get_nc():
    global _NC_CACHE
    if _NC_CACHE is None:
        _NC_CACHE = build_nc()
    return _NC_CACHE


def make_in_maps(x, cos, sin, Wq, Wk, Wv, Wo, q_norm_w, k_norm_w):
    x = np.asarray(x, dtype=np.float32).reshape(T, HID)
    xt = np.ascontiguousarray(x.T)
    cosT = np.ascontiguousarray(np.asarray(cos, np.float32).T)
    # rolled by 64: sinT_roll[d] = sin.T[(d - 64) % 128]
    sinT = np.ascontiguousarray(
        np.roll(np.asarray(sin, np.float32).T, 64, axis=0)
    )
    qw = np.ascontiguousarray(np.asarray(q_norm_w, np.float32).reshape(D, 1))
    kw = np.ascontiguousarray(np.asarray(k_norm_w, np.float32).reshape(D, 1))
    si = np.arange(P)[:, None, None]
    jj = np.arange(4)[None, :, None]
    qi = np.arange(512)[None, None, :]
    masks = (si + P * jj <= qi).astype(np.float32)
    Wq = np.asarray(Wq, np.float32)
    Wk = np.asarray(Wk, np.float32)
    Wv = np.asarray(Wv, np.float32)
    Wo = np.asarray(Wo, np.float32)
    in_maps = []
    for c in range(N_CORES):
        in_maps.append({
            "xt": xt,
            "wq": np.ascontiguousarray(Wq[:, c * HQ * D:(c + 1) * HQ * D]),
            "wk": np.ascontiguousarray(Wk[:, c * D:(c + 1) * D]),
            "wv": np.ascontiguousarray(Wv[:, c * D:(c + 1) * D]),
            "wo": np.ascontiguousarray(Wo[c * HQ * D:(c + 1) * HQ * D, :]),
            "cosT": cosT,
            "sinT": sinT,
            "qw": qw,
            "kw": kw,
            "masks": masks,
        })
    return in_maps


def kernel(x, cos, sin, Wq, Wk, Wv, Wo, q_norm_w, k_norm_w):
    nc = get_nc()
    in_maps = make_in_maps(x, cos, sin, Wq, Wk, Wv, Wo, q_norm_w, k_norm_w)
    res = run_bass_kernel_spmd(nc, in_maps, core_ids=list(range(N_CORES)))
    acc = np.zeros((T, HID), dtype=np.float32)
    for c in range(N_CORES):
        acc += res.results[c]["out"]
    return acc.reshape(1, T, HID)
